# revision 11
# baseline (speedup 1.0000x reference)
"""Multi-head attention (B=2, S=2048, D=1024, H=16, hd=64, RoPE, causal)
on 8 Trainium2 NeuronCores.

Sharding: each core owns 2 heads x both batches (tensor-parallel over heads).
Per core, everything is computed in transposed [feature, seq] layout with
f32r matmuls:
  - Q/K/V projections from pre-transposed x (QT/KT/VT = W.T-slice.T @ x.T)
  - RoPE on QT/KT via a partition-swap (SBUF->SBUF DMA) + 3 DVE ops
  - scores computed TRANSPOSED: ST[k,q] = KT_h.T @ QT_h, so softmax needs no
    max-subtraction (scores bounded by ~+-4) and no P-transpose; causal
    handled by loop bounds + one static triangle tile on diagonal blocks
  - exp on ACT with fused 1/sqrt(hd) scale; denominator via a ones-column
    appended to V (65th lane of the attn@V accumulation)
  - re-shard heads->sequence via an 8-core AllToAll (256KB/core shards);
    each core then computes a disjoint out-projection quarter (512 seq
    positions), so the host only concatenates/transposes.
"""
import os

import numpy as np

import concourse.bass as bass
import concourse.mybir as mybir
import concourse.tile as tile
from concourse.bass_utils import run_bass_kernel_spmd
from concourse.vector_clock import ScopedClock

B, S, D, H, HD = 2, 2048, 1024, 16, 64
NCORES = 8
HPC = 2                    # heads per core
F = HPC * HD               # 128 features per core
CHUNK = 512
NCH = S // CHUNK           # 4 q-chunks
NKT = D // 128             # 8 contraction tiles for projections
NST = S // 128             # 16 key tiles
MASKVAL = -240.0           # -30 after the 1/8 softmax scale; exp(-30) ~ 1e-13
F32 = mybir.dt.float32
F32R = mybir.dt.float32r


# ---------------------------------------------------------------------------
# Workarounds for the walrus build in this container: it encodes at most ONE
# sync-wait per instruction ("Too many sync wait commands"). Split multi-wait
# instructions into single-wait NoOps. Semantics-preserving.
# ---------------------------------------------------------------------------
_patched = False


def _install_patches():
    global _patched
    if _patched:
        return
    _patched = True

    _orig_lower = tile.TileContext._lower_ordered_insts

    def _lower_with_wait_split(self, ordered):
        nc = self.nc
        for _bb, insts in ordered.items():
            if not any(
                i.sync_info is not None and len(i.sync_info.on_wait) > 1
                for i in insts
            ):
                continue
            new = []
            for inst in insts:
                si = inst.sync_info
                if si is not None and len(si.on_wait) > 1:
                    waits = list(si.on_wait)
                    for w in waits[:-1]:
                        n = mybir.InstNoOp(
                            name=f"I-waitsplit-{nc.next_id()}", ins=[], outs=[]
                        )
                        n.engine = inst.engine
                        n.bass_nofuse = True
                        n.sync_info = mybir.SyncInfo(on_wait=[w], on_update=[])
                        nc.register_instruction(n)
                        new.append(n)
                    inst.sync_info = mybir.SyncInfo(
                        on_wait=[waits[-1]], on_update=list(si.on_update)
                    )
                new.append(inst)
            insts[:] = new
        return _orig_lower(self, ordered)

    tile.TileContext._lower_ordered_insts = _lower_with_wait_split

    def _drain_and_barrier(self, tick_clock, wait_clock):
        nc = self.nc
        probe = nc.sync.nop(nofuse=True)
        wait_clock.add_sem_waits(
            probe.ins, ScopedClock({None: tick_clock.global_clock})
        )
        waits = list(probe.ins.sync_info.on_wait)
        probe.ins.sync_info = mybir.SyncInfo(on_wait=waits[:1], on_update=[])
        for w in waits[1:]:
            n2 = nc.sync.nop(nofuse=True)
            n2.ins.sync_info = mybir.SyncInfo(on_wait=[w], on_update=[])
        nc.sync.drain()
        nc.all_engine_barrier()
        assert self.sems is not None
        popped = nc._tile_sem_poison_stack.pop()
        assert popped is self._sem_poison
        nc.clear_and_free_semaphores(list(self.sems.allocated().values()))
        nc.all_engine_barrier()

    tile.TileContext._drain_and_barrier = _drain_and_barrier


def _install_ntff_hook():
    """Provide the missing ``antenv.axon_hooks`` module so trace=True works."""
    import sys
    import types

    if "antenv.axon_hooks" in sys.modules:
        return
    try:
        import antenv
        from trn_agent_boot.trn_boot import _ntff_profile_via_ctypes
    except ImportError:
        return
    mod = types.ModuleType("antenv.axon_hooks")
    mod._hook = _ntff_profile_via_ctypes("/opt/axon/libaxon_pjrt.so")
    mod.set_axon_ntff_profile_hook = lambda h: setattr(mod, "_hook", h)
    mod.get_axon_ntff_profile_hook = lambda: mod._hook
    sys.modules["antenv.axon_hooks"] = mod
    antenv.axon_hooks = mod


def _r(ap):
    """View an fp32 AP as f32r for full-rate PE matmuls."""
    return ap.bitcast(F32R)


# ---------------------------------------------------------------------------
# Program builder (same program on all 8 cores; per-core data differs)
# ---------------------------------------------------------------------------
def build_program():
    _install_patches()
    nc = bass.Bass(num_devices=NCORES)

    xt = [nc.dram_tensor(f"xt{b}", [D, S], F32R, kind="ExternalInput")
          for b in range(B)]
    wqt = nc.dram_tensor("wqt", [D, F], F32R, kind="ExternalInput")
    wkt = nc.dram_tensor("wkt", [D, F], F32R, kind="ExternalInput")
    wvt = nc.dram_tensor("wvt", [D, F], F32R, kind="ExternalInput")
    bq = nc.dram_tensor("bq", [F], F32, kind="ExternalInput")
    bk = nc.dram_tensor("bk", [F], F32, kind="ExternalInput")
    bv = nc.dram_tensor("bv", [F], F32, kind="ExternalInput")
    wot = nc.dram_tensor("wot", [D, D], F32R, kind="ExternalInput")
    bo = nc.dram_tensor("bo", [D], F32, kind="ExternalInput")
    chat = nc.dram_tensor("chat", [F, S], F32, kind="ExternalInput")
    ident_in = nc.dram_tensor("ident128", [128, 128], F32R, kind="ExternalInput")
    ones_in = nc.dram_tensor("ones64", [1, 64], F32R, kind="ExternalInput")
    vones_in = nc.dram_tensor("vones", [NST, HPC], F32R, kind="ExternalInput")
    mask_in = nc.dram_tensor("mask128", [128, 128], F32, kind="ExternalInput")
    shat = nc.dram_tensor("shat", [F, S], F32, kind="ExternalInput")
    ytq = nc.dram_tensor("ytq", [D, CHUNK], F32, kind="ExternalOutput")

    a2a_in = nc.dram_tensor("a2a_in", [NCORES, F * CHUNK], F32R)
    a2a_out = nc.dram_tensor("a2a_out", [NCORES, F * CHUNK], F32R)
    a2a_in3 = a2a_in.rearrange("g (p n) -> g p n", p=F)
    a2a_out3 = a2a_out.rearrange("g (p n) -> g p n", p=F)

    with tile.TileContext(nc) as tc:
        with (
            tc.tile_pool(name="const", bufs=1) as const,
            tc.tile_pool(name="wpool", bufs=1) as wpool,
            tc.tile_pool(name="xtp", bufs=2) as xtp,
            tc.tile_pool(name="raw", bufs=3) as raw,
            tc.tile_pool(name="ropetmp", bufs=2) as ropetmp,
            tc.tile_pool(name="qkv", bufs=1) as qkv,
            tc.tile_pool(name="vagg", bufs=1) as vaggp,
            tc.tile_pool(name="expp", bufs=4) as expp,
            tc.tile_pool(name="normp", bufs=2) as normp,
            tc.tile_pool(name="stage", bufs=4) as stage,
            tc.tile_pool(name="at2", bufs=1) as at2p,
            tc.tile_pool(name="ys", bufs=2) as ysp,
            tc.tile_pool(name="ps", bufs=3, space="PSUM") as ps,
            tc.tile_pool(name="pav", bufs=2, space="PSUM") as pav,
        ):
            # ---- constants ----
            ident = const.tile([128, 128], F32R)
            nc.sync.dma_start(out=ident, in_=ident_in[:])
            mask = const.tile([128, 128], F32)
            nc.sync.dma_start(out=mask, in_=mask_in[:])
            ones_t = const.tile([1, 64], F32R)
            nc.sync.dma_start(out=ones_t, in_=ones_in[:])
            chat_t = const.tile([F, S], F32)
            nc.sync.dma_start(out=chat_t, in_=chat[:])
            shat_t = const.tile([F, S], F32)
            nc.sync.dma_start(out=shat_t, in_=shat[:])
            bq_t = const.tile([F, 1], F32)
            nc.sync.dma_start(out=bq_t, in_=bq.rearrange("(p o) -> p o", o=1))
            bk_t = const.tile([F, 1], F32)
            nc.sync.dma_start(out=bk_t, in_=bk.rearrange("(p o) -> p o", o=1))
            bv_t = const.tile([F, 1], F32)
            nc.sync.dma_start(out=bv_t, in_=bv.rearrange("(p o) -> p o", o=1))
            bo_t = const.tile([128, NKT], F32)
            nc.sync.dma_start(out=bo_t, in_=bo.rearrange("(e p) -> p e", p=128))

            wq_t = [wpool.tile([128, F], F32R, tag=f"wq{k}", name=f"wq{k}") for k in range(NKT)]
            wk_t = [wpool.tile([128, F], F32R, tag=f"wk{k}", name=f"wk{k}") for k in range(NKT)]
            wv_t = [wpool.tile([128, F], F32R, tag=f"wv{k}", name=f"wv{k}") for k in range(NKT)]
            for k in range(NKT):
                nc.sync.dma_start(out=wq_t[k], in_=wqt[128*k:128*(k+1), :])
                nc.sync.dma_start(out=wk_t[k], in_=wkt[128*k:128*(k+1), :])
                nc.sync.dma_start(out=wv_t[k], in_=wvt[128*k:128*(k+1), :])
            wo_t = [wpool.tile([128, D], F32R, tag=f"wo{k}", name=f"wo{k}") for k in range(NKT)]
            for k in range(NKT):
                nc.sync.dma_start(out=wo_t[k], in_=wot[128*k:128*(k+1), :])

            # ---- per batch: projections + rope + attention ----
            for b in range(B):
                QT = qkv.tile([F, S], F32R, tag="QT")
                KT = qkv.tile([F, S], F32R, tag="KT")
                VT = qkv.tile([F, S], F32R, tag="VT")
                vagg = vaggp.tile([128, NST, HPC * 65], F32R)
                # ones columns of the V augmentation, broadcast from DRAM
                vi = vones_in[:]
                vones_bcast = bass.AP(
                    tensor=vi.tensor, offset=vi.offset,
                    ap=[[0, 128]] + [list(p) for p in vi.ap],
                )
                nc.sync.dma_start(
                    out=vagg.rearrange("p st (h u) -> p st h u", u=65)
                        [:, :, :, 64],
                    in_=vones_bcast,
                )

                for c in range(NCH):
                    cs = slice(CHUNK * c, CHUNK * (c + 1))
                    xt_c = [xtp.tile([128, CHUNK], F32R, name=f"xt_c{k2}") for k2 in range(NKT)]
                    for k in range(NKT):
                        nc.sync.dma_start(
                            out=xt_c[k], in_=xt[b][128*k:128*(k+1), cs]
                        )
                    for name, w_t, b_t, dst in (
                        ("q", wq_t, bq_t, QT),
                        ("k", wk_t, bk_t, KT),
                        ("v", wv_t, bv_t, VT),
                    ):
                        pm = ps.tile([F, CHUNK], F32, tag="mm", name="pm_proj")
                        for k in range(NKT):
                            nc.tensor.matmul(
                                pm, w_t[k], xt_c[k],
                                start=(k == 0), stop=(k == NKT - 1),
                            )
                        if name == "v":
                            # bias folded here; no rope for V
                            nc.scalar.activation(
                                VT[:, cs], pm,
                                mybir.ActivationFunctionType.Identity,
                                bias=b_t[:],
                            )
                        else:
                            rawt = raw.tile([F, CHUNK], F32, tag="rawqk")
                            nc.scalar.activation(
                                rawt, pm,
                                mybir.ActivationFunctionType.Identity,
                                bias=b_t[:],
                            )
                            # rope: dst = raw*Chat + swap32(raw)*Shat
                            swp = raw.tile([F, CHUNK], F32, tag="swp")
                            nc.sync.dma_start(out=swp[0:32, :], in_=rawt[32:64, :])
                            nc.sync.dma_start(out=swp[32:64, :], in_=rawt[0:32, :])
                            nc.sync.dma_start(out=swp[64:96, :], in_=rawt[96:128, :])
                            nc.sync.dma_start(out=swp[96:128, :], in_=rawt[64:96, :])
                            t1 = ropetmp.tile([F, CHUNK], F32, tag="t1")
                            nc.vector.tensor_mul(t1, rawt, chat_t[:, cs])
                            t2 = ropetmp.tile([F, CHUNK], F32, tag="t2")
                            nc.vector.tensor_mul(t2, swp, shat_t[:, cs])
                            nc.vector.tensor_add(dst[:, cs], t1, t2)

                    # V transpose for this chunk's 4 s-tiles into vagg
                    for st in range(4 * c, 4 * c + 4):
                        pt = ps.tile([128, 128], F32R, tag="vtr", bufs=1, name="pt_vtr")
                        nc.tensor.transpose(
                            pt, VT[:, 128*st:128*(st+1)], ident[:]
                        )
                        nc.scalar.activation(
                            vagg.rearrange("p st (h u) -> p st h u", u=65)
                                [:, st, :, 0:64],
                            pt.rearrange("p (h u) -> p h u", h=HPC),
                            mybir.ActivationFunctionType.Copy,
                        )

                # attention: transposed scores, per head / q-chunk / k-tile
                for h in range(HPC):
                    hs = slice(64 * h, 64 * (h + 1))
                    for c in range(NCH):
                        av = pav.tile([65, CHUNK], F32, tag="av")
                        for kt in range(4 * c + 4):
                            qlo = max(CHUNK * c, 128 * kt)
                            w = CHUNK * (c + 1) - qlo
                            pm = ps.tile([128, CHUNK], F32, tag="mm", name="pm_scores")
                            nc.tensor.matmul(
                                pm[:, 0:w],
                                KT[hs, 128*kt:128*(kt+1)],
                                QT[hs, qlo:qlo + w],
                                start=True, stop=True,
                            )
                            if 128 * kt >= CHUNK * c:
                                nc.vector.tensor_add(
                                    pm[:, 0:128], pm[:, 0:128], mask[:]
                                )
                            ex = expp.tile([128, CHUNK], F32R, tag="exp")
                            nc.scalar.activation(
                                ex[:, 0:w], pm[:, 0:w],
                                mybir.ActivationFunctionType.Exp,
                                scale=0.125,
                            )
                            off = qlo - CHUNK * c
                            nc.tensor.matmul(
                                av[:, off:CHUNK],
                                vagg[:, kt, 65*h:65*(h+1)],
                                ex[:, 0:w],
                                start=(kt == 0), stop=(kt == 4 * c + 3),
                                skip_group_check=True,
                            )
                        # normalize: attnT = av[0:64] * (1/av[64]) bcast
                        recs = normp.tile([65, CHUNK], F32, tag="recs")
                        nc.vector.reciprocal(recs[64:65, :], av[64:65, :])
                        recl = normp.tile([1, CHUNK], F32R, tag="recl")
                        nc.sync.dma_start(out=recl, in_=_r(recs[64:65, :]))
                        pb = pav.tile([64, CHUNK], F32, tag="pb", bufs=1)
                        nc.tensor.matmul(pb, ones_t[:], recl[:],
                                         start=True, stop=True)
                        recb = normp.tile([64, CHUNK], F32, tag="recb")
                        nc.scalar.activation(
                            recb, pb, mybir.ActivationFunctionType.Copy)
                        sg = stage.tile([64, CHUNK], F32, tag="sg")
                        nc.vector.tensor_mul(sg, av[0:64, :], recb[:])
                        nc.sync.dma_start(
                            out=a2a_in3[4 * b + c][hs, :], in_=_r(sg)
                        )

            # ---- all-to-all: heads -> sequence quarters ----
            nc.gpsimd.collective_compute(
                "AllToAll",
                mybir.AluOpType.bypass,
                replica_groups=[list(range(NCORES))],
                ins=[a2a_in[:]],
                outs=[a2a_out[:]],
            )

            # ---- out projection for my sequence quarter ----
            at2 = [at2p.tile([128, CHUNK], F32R, tag=f"at{g}", name=f"at{g}") for g in range(NCORES)]
            for g in range(NCORES):
                nc.sync.dma_start(out=at2[g], in_=a2a_out3[g])
            for et in range(NKT):
                pm = ps.tile([128, CHUNK], F32, tag="mm", name="pm_yproj")
                for k in range(NKT):
                    nc.tensor.matmul(
                        pm, wo_t[k][:, 128*et:128*(et+1)], at2[k],
                        start=(k == 0), stop=(k == NKT - 1),
                    )
                ys = ysp.tile([128, CHUNK], F32, tag="ys")
                nc.scalar.activation(
                    ys, pm, mybir.ActivationFunctionType.Identity,
                    bias=bo_t[:, et:et+1],
                )
                nc.sync.dma_start(out=ytq[128*et:128*(et+1), :], in_=ys)

    nc.finalize()
    return nc


_NC_CACHE = None


def _get_program():
    global _NC_CACHE
    if _NC_CACHE is None:
        _NC_CACHE = build_program()
    return _NC_CACHE


def _prep_in_maps(x, cos, sin, Wq, bq, Wk, bk, Wv, bv, Wo, bo):
    cosT = np.ascontiguousarray(cos.T).astype(np.float32)    # (32, S)
    sinT = np.ascontiguousarray(sin.T).astype(np.float32)
    chat = np.concatenate([cosT, cosT, cosT, cosT], 0)       # (128, S)
    shat = np.concatenate([-sinT, sinT, -sinT, sinT], 0)
    xT = [np.ascontiguousarray(x[b].T) for b in range(B)]
    mask128 = np.where(np.arange(128)[:, None] > np.arange(128)[None, :],
                       np.float32(MASKVAL), np.float32(0.0)).astype(np.float32)
    wqT, wkT, wvT, woT = (np.ascontiguousarray(W.T) for W in (Wq, Wk, Wv, Wo))

    in_maps = []
    for core in range(NCORES):
        sl = slice(F * core, F * (core + 1))
        in_maps.append({
            "xt0": xT[0], "xt1": xT[1],
            "wqt": np.ascontiguousarray(wqT[:, sl]),
            "wkt": np.ascontiguousarray(wkT[:, sl]),
            "wvt": np.ascontiguousarray(wvT[:, sl]),
            "bq": np.ascontiguousarray(bq[sl]),
            "bk": np.ascontiguousarray(bk[sl]),
            "bv": np.ascontiguousarray(bv[sl]),
            "wot": woT, "bo": bo,
            "chat": chat, "shat": shat,
            "ident128": np.eye(128, dtype=np.float32),
            "ones64": np.ones((1, 64), np.float32),
            "vones": np.ones((NST, HPC), np.float32),
            "mask128": mask128,
        })
    return in_maps


def kernel(x, cos, sin, mask, Wq, bq, Wk, bk, Wv, bv, Wo, bo, **_unused):
    """Full inputs in, full output out. `mask` (the causal mask) is
    regenerated on-device, so the input tensor itself is unused."""
    x, cos, sin = (np.asarray(a, np.float32) for a in (x, cos, sin))
    Wq, bq, Wk, bk = (np.asarray(a, np.float32) for a in (Wq, bq, Wk, bk))
    Wv, bv, Wo, bo = (np.asarray(a, np.float32) for a in (Wv, bv, Wo, bo))

    nc = _get_program()
    in_maps = _prep_in_maps(x, cos, sin, Wq, bq, Wk, bk, Wv, bv, Wo, bo)

    trace = bool(int(os.environ.get("MHA_TRACE", "0")))
    kw = {}
    if trace:
        _install_ntff_hook()
        kw = dict(trace=True, trace_cores=list(range(NCORES)))
    res = run_bass_kernel_spmd(nc, in_maps, core_ids=list(range(NCORES)), **kw)
    kernel.last_results = res

    y = np.empty((B, S, D), np.float32)
    for r in range(NCORES):
        b, c = r // NCH, r % NCH
        y[b, CHUNK*c:CHUNK*(c+1), :] = res.results[r]["ytq"].T
    return y


# revision 15
# speedup vs baseline: 1.1771x; 1.1771x over previous
"""Multi-head attention (B=2, S=2048, D=1024, H=16, hd=64, RoPE, causal)
on 8 Trainium2 NeuronCores.

Sharding: each core owns 2 heads x both batches (tensor-parallel over heads).
Per core, everything is computed in transposed [feature, seq] layout with
f32r matmuls:
  - Q/K/V projections from pre-transposed x (QT/KT/VT = W.T-slice.T @ x.T)
  - RoPE on QT/KT via a partition-swap (SBUF->SBUF DMA) + 3 DVE ops
  - scores computed TRANSPOSED: ST[k,q] = KT_h.T @ QT_h, so softmax needs no
    max-subtraction (scores bounded by ~+-4) and no P-transpose; causal
    handled by loop bounds + one static triangle tile on diagonal blocks
  - exp on ACT with fused 1/sqrt(hd) scale; denominator via a ones-column
    appended to V (65th lane of the attn@V accumulation)
  - re-shard heads->sequence via an 8-core AllToAll (256KB/core shards);
    each core then computes a disjoint out-projection quarter (512 seq
    positions), so the host only concatenates/transposes.
"""
import os

import numpy as np

import concourse.bass as bass
import concourse.mybir as mybir
import concourse.tile as tile
from concourse.bass_utils import run_bass_kernel_spmd
from concourse.vector_clock import ScopedClock

B, S, D, H, HD = 2, 2048, 1024, 16, 64
NCORES = 8
HPC = 2                    # heads per core
F = HPC * HD               # 128 features per core
CHUNK = 512
NCH = S // CHUNK           # 4 q-chunks
NKT = D // 128             # 8 contraction tiles for projections
NST = S // 128             # 16 key tiles
MASKVAL = -240.0           # -30 after the 1/8 softmax scale; exp(-30) ~ 1e-13
F32 = mybir.dt.float32
F32R = mybir.dt.float32r


# ---------------------------------------------------------------------------
# Workarounds for the walrus build in this container: it encodes at most ONE
# sync-wait per instruction ("Too many sync wait commands"). Split multi-wait
# instructions into single-wait NoOps. Semantics-preserving.
# ---------------------------------------------------------------------------
_patched = False


def _install_patches():
    global _patched
    if _patched:
        return
    _patched = True

    _orig_lower = tile.TileContext._lower_ordered_insts

    def _lower_with_wait_split(self, ordered):
        nc = self.nc
        for _bb, insts in ordered.items():
            if not any(
                i.sync_info is not None and len(i.sync_info.on_wait) > 1
                for i in insts
            ):
                continue
            new = []
            for inst in insts:
                si = inst.sync_info
                if si is not None and len(si.on_wait) > 1:
                    waits = list(si.on_wait)
                    for w in waits[:-1]:
                        n = mybir.InstNoOp(
                            name=f"I-waitsplit-{nc.next_id()}", ins=[], outs=[]
                        )
                        n.engine = inst.engine
                        n.bass_nofuse = True
                        n.sync_info = mybir.SyncInfo(on_wait=[w], on_update=[])
                        nc.register_instruction(n)
                        new.append(n)
                    inst.sync_info = mybir.SyncInfo(
                        on_wait=[waits[-1]], on_update=list(si.on_update)
                    )
                new.append(inst)
            insts[:] = new
        return _orig_lower(self, ordered)

    tile.TileContext._lower_ordered_insts = _lower_with_wait_split

    def _drain_and_barrier(self, tick_clock, wait_clock):
        nc = self.nc
        probe = nc.sync.nop(nofuse=True)
        wait_clock.add_sem_waits(
            probe.ins, ScopedClock({None: tick_clock.global_clock})
        )
        waits = list(probe.ins.sync_info.on_wait)
        probe.ins.sync_info = mybir.SyncInfo(on_wait=waits[:1], on_update=[])
        for w in waits[1:]:
            n2 = nc.sync.nop(nofuse=True)
            n2.ins.sync_info = mybir.SyncInfo(on_wait=[w], on_update=[])
        nc.sync.drain()
        nc.all_engine_barrier()
        assert self.sems is not None
        popped = nc._tile_sem_poison_stack.pop()
        assert popped is self._sem_poison
        nc.clear_and_free_semaphores(list(self.sems.allocated().values()))
        nc.all_engine_barrier()

    tile.TileContext._drain_and_barrier = _drain_and_barrier


def _install_ntff_hook():
    """Provide the missing ``antenv.axon_hooks`` module so trace=True works."""
    import sys
    import types

    if "antenv.axon_hooks" in sys.modules:
        return
    try:
        import antenv
        from trn_agent_boot.trn_boot import _ntff_profile_via_ctypes
    except ImportError:
        return
    mod = types.ModuleType("antenv.axon_hooks")
    mod._hook = _ntff_profile_via_ctypes("/opt/axon/libaxon_pjrt.so")
    mod.set_axon_ntff_profile_hook = lambda h: setattr(mod, "_hook", h)
    mod.get_axon_ntff_profile_hook = lambda: mod._hook
    sys.modules["antenv.axon_hooks"] = mod
    antenv.axon_hooks = mod


def _r(ap):
    """View an fp32 AP as f32r for full-rate PE matmuls."""
    return ap.bitcast(F32R)


# ---------------------------------------------------------------------------
# Program builder (same program on all 8 cores; per-core data differs)
# ---------------------------------------------------------------------------
def build_program():
    _install_patches()
    nc = bass.Bass(num_devices=NCORES)

    xt = [nc.dram_tensor(f"xt{b}", [D, S], F32R, kind="ExternalInput")
          for b in range(B)]
    wqt = nc.dram_tensor("wqt", [D, F], F32R, kind="ExternalInput")
    wkt = nc.dram_tensor("wkt", [D, F], F32R, kind="ExternalInput")
    wvt = nc.dram_tensor("wvt", [D, F], F32R, kind="ExternalInput")
    bq = nc.dram_tensor("bq", [F], F32, kind="ExternalInput")
    bk = nc.dram_tensor("bk", [F], F32, kind="ExternalInput")
    bv = nc.dram_tensor("bv", [F], F32, kind="ExternalInput")
    wot = nc.dram_tensor("wot", [D, D], F32R, kind="ExternalInput")
    bo = nc.dram_tensor("bo", [D], F32, kind="ExternalInput")
    chat = nc.dram_tensor("chat", [F, S], F32, kind="ExternalInput")
    ident_in = nc.dram_tensor("ident128", [128, 128], F32R, kind="ExternalInput")
    perm_in = nc.dram_tensor("perm128", [128, 128], F32R, kind="ExternalInput")
    ones_in = nc.dram_tensor("ones64", [1, 64], F32R, kind="ExternalInput")
    vones_in = nc.dram_tensor("vones", [NST, HPC], F32R, kind="ExternalInput")
    mask_in = nc.dram_tensor("mask128", [128, 128], F32, kind="ExternalInput")
    shat = nc.dram_tensor("shat", [F, S], F32, kind="ExternalInput")
    ytq = nc.dram_tensor("ytq", [D, CHUNK], F32, kind="ExternalOutput")

    a2a_in = nc.dram_tensor("a2a_in", [NCORES, F * CHUNK], F32R)
    a2a_out = nc.dram_tensor("a2a_out", [NCORES, F * CHUNK], F32R)
    a2a_in3 = a2a_in.rearrange("g (p n) -> g p n", p=F)
    a2a_out3 = a2a_out.rearrange("g (p n) -> g p n", p=F)

    with tile.TileContext(nc) as tc:
        with (
            tc.tile_pool(name="const", bufs=1) as const,
            tc.tile_pool(name="wpool", bufs=1) as wpool,
            tc.tile_pool(name="xtp", bufs=2) as xtp,
            tc.tile_pool(name="raw", bufs=3) as raw,
            tc.tile_pool(name="ropetmp", bufs=2) as ropetmp,
            tc.tile_pool(name="qkv", bufs=1) as qkv,
            tc.tile_pool(name="vagg", bufs=1) as vaggp,
            tc.tile_pool(name="expp", bufs=4) as expp,
            tc.tile_pool(name="normp", bufs=2) as normp,
            tc.tile_pool(name="stage", bufs=4) as stage,
            tc.tile_pool(name="at2", bufs=1) as at2p,
            tc.tile_pool(name="ys", bufs=2) as ysp,
            tc.tile_pool(name="ps", bufs=3, space="PSUM") as ps,
            tc.tile_pool(name="pav", bufs=2, space="PSUM") as pav,
        ):
            # ---- constants ----
            ident = const.tile([128, 128], F32R)
            nc.sync.dma_start(out=ident, in_=ident_in[:])
            perm = const.tile([128, 128], F32R)
            nc.sync.dma_start(out=perm, in_=perm_in[:])
            mask = const.tile([128, 128], F32)
            nc.sync.dma_start(out=mask, in_=mask_in[:])
            ones_t = const.tile([65, 64], F32R)
            nc.sync.dma_start(out=ones_t[64:65, :], in_=ones_in[:])
            chat_t = const.tile([F, S], F32)
            nc.sync.dma_start(out=chat_t, in_=chat[:])
            shat_t = const.tile([F, S], F32)
            nc.sync.dma_start(out=shat_t, in_=shat[:])
            bq_t = const.tile([F, 1], F32)
            nc.sync.dma_start(out=bq_t, in_=bq.rearrange("(p o) -> p o", o=1))
            bk_t = const.tile([F, 1], F32)
            nc.sync.dma_start(out=bk_t, in_=bk.rearrange("(p o) -> p o", o=1))
            bv_t = const.tile([F, 1], F32)
            nc.sync.dma_start(out=bv_t, in_=bv.rearrange("(p o) -> p o", o=1))
            bo_t = const.tile([128, NKT], F32)
            nc.sync.dma_start(out=bo_t, in_=bo.rearrange("(e p) -> p e", p=128))

            wq_t = [wpool.tile([128, F], F32R, tag=f"wq{k}", name=f"wq{k}") for k in range(NKT)]
            wk_t = [wpool.tile([128, F], F32R, tag=f"wk{k}", name=f"wk{k}") for k in range(NKT)]
            wv_t = [wpool.tile([128, F], F32R, tag=f"wv{k}", name=f"wv{k}") for k in range(NKT)]
            for k in range(NKT):
                nc.sync.dma_start(out=wq_t[k], in_=wqt[128*k:128*(k+1), :])
                nc.sync.dma_start(out=wk_t[k], in_=wkt[128*k:128*(k+1), :])
                nc.sync.dma_start(out=wv_t[k], in_=wvt[128*k:128*(k+1), :])
            wo_t = [wpool.tile([128, D], F32R, tag=f"wo{k}", name=f"wo{k}") for k in range(NKT)]
            for k in range(NKT):
                nc.sync.dma_start(out=wo_t[k], in_=wot[128*k:128*(k+1), :])

            # ---- per batch: projections + rope + attention ----
            for b in range(B):
                QT = qkv.tile([F, S], F32R, tag="QT")
                KT = qkv.tile([F, S], F32R, tag="KT")
                VT = qkv.tile([F, S], F32R, tag="VT")
                vagg = vaggp.tile([128, NST, HPC * 65], F32R)
                # ones columns of the V augmentation, broadcast from DRAM
                vi = vones_in[:]
                vones_bcast = bass.AP(
                    tensor=vi.tensor, offset=vi.offset,
                    ap=[[0, 128]] + [list(p) for p in vi.ap],
                )
                nc.sync.dma_start(
                    out=vagg.rearrange("p st (h u) -> p st h u", u=65)
                        [:, :, :, 64],
                    in_=vones_bcast,
                )

                for c in range(NCH):
                    cs = slice(CHUNK * c, CHUNK * (c + 1))
                    xt_c = [xtp.tile([128, CHUNK], F32R, name=f"xt_c{k2}") for k2 in range(NKT)]
                    for k in range(NKT):
                        nc.sync.dma_start(
                            out=xt_c[k], in_=xt[b][128*k:128*(k+1), cs]
                        )
                    for name, w_t, b_t, dst in (
                        ("q", wq_t, bq_t, QT),
                        ("k", wk_t, bk_t, KT),
                        ("v", wv_t, bv_t, VT),
                    ):
                        pm = ps.tile([F, CHUNK], F32, tag="mm", name="pm_proj")
                        for k in range(NKT):
                            nc.tensor.matmul(
                                pm, w_t[k], xt_c[k],
                                start=(k == 0), stop=(k == NKT - 1),
                            )
                        if name == "v":
                            # bias folded here; no rope for V
                            nc.scalar.activation(
                                VT[:, cs], pm,
                                mybir.ActivationFunctionType.Identity,
                                bias=b_t[:],
                            )
                        else:
                            rawt = raw.tile([F, CHUNK], F32R, tag="rawqk")
                            nc.scalar.activation(
                                rawt, pm,
                                mybir.ActivationFunctionType.Identity,
                                bias=b_t[:],
                            )
                            # rope: dst = raw*Chat + swap32(raw)*Shat,
                            # swap32 done as a PE permutation matmul
                            psw = ps.tile([F, CHUNK], F32, tag="mm", name="psw")
                            nc.tensor.matmul(psw, perm, rawt,
                                             start=True, stop=True)
                            t1 = ropetmp.tile([F, CHUNK], F32, tag="t1")
                            nc.vector.tensor_mul(t1, rawt.bitcast(F32),
                                                 chat_t[:, cs])
                            t2 = ropetmp.tile([F, CHUNK], F32, tag="t2")
                            nc.vector.tensor_mul(t2, psw, shat_t[:, cs])
                            nc.vector.tensor_add(dst[:, cs], t1, t2)

                    # V transpose for this chunk's 4 s-tiles into vagg
                    for st in range(4 * c, 4 * c + 4):
                        pt = ps.tile([128, 128], F32R, tag="vtr", bufs=1, name="pt_vtr")
                        nc.tensor.transpose(
                            pt, VT[:, 128*st:128*(st+1)], ident[:]
                        )
                        nc.scalar.activation(
                            vagg.rearrange("p st (h u) -> p st h u", u=65)
                                [:, st, :, 0:64],
                            pt.rearrange("p (h u) -> p h u", h=HPC),
                            mybir.ActivationFunctionType.Copy,
                        )

                # attention: transposed scores, per head / q-chunk / k-tile
                for h in range(HPC):
                    hs = slice(64 * h, 64 * (h + 1))
                    for c in range(NCH):
                        av = pav.tile([65, CHUNK], F32, tag="av")
                        for kt in range(4 * c + 4):
                            qlo = max(CHUNK * c, 128 * kt)
                            w = CHUNK * (c + 1) - qlo
                            pm = ps.tile([128, CHUNK], F32, tag="mm", name="pm_scores")
                            nc.tensor.matmul(
                                pm[:, 0:w],
                                KT[hs, 128*kt:128*(kt+1)],
                                QT[hs, qlo:qlo + w],
                                start=True, stop=True,
                            )
                            if 128 * kt >= CHUNK * c:
                                nc.vector.tensor_add(
                                    pm[:, 0:128], pm[:, 0:128], mask[:]
                                )
                            ex = expp.tile([128, CHUNK], F32R, tag="exp")
                            nc.scalar.activation(
                                ex[:, 0:w], pm[:, 0:w],
                                mybir.ActivationFunctionType.Exp,
                                scale=0.125,
                            )
                            off = qlo - CHUNK * c
                            nc.tensor.matmul(
                                av[:, off:CHUNK],
                                vagg[:, kt, 65*h:65*(h+1)],
                                ex[:, 0:w],
                                start=(kt == 0), stop=(kt == 4 * c + 3),
                                skip_group_check=True,
                            )
                        # normalize: attnT = av[0:64] / bcast(av[64])
                        denl = normp.tile([65, CHUNK], F32R, tag="denl")
                        nc.scalar.activation(
                            denl[64:65, :], av[64:65, :],
                            mybir.ActivationFunctionType.Copy)
                        pb = pav.tile([64, CHUNK], F32, tag="pb", bufs=1)
                        nc.tensor.matmul(pb, ones_t[64:65, :], denl[64:65, :],
                                         start=True, stop=True)
                        # 1/denom via exp(-ln(denom)) on the ACT LUTs
                        lnb = normp.tile([64, CHUNK], F32, tag="lnb")
                        nc.scalar.activation(
                            lnb, pb, mybir.ActivationFunctionType.Ln)
                        recb2 = normp.tile([64, CHUNK], F32, tag="recb2")
                        nc.scalar.activation(
                            recb2, lnb, mybir.ActivationFunctionType.Exp,
                            scale=-1.0)
                        sg = stage.tile([64, CHUNK], F32, tag="sg")
                        nc.vector.tensor_mul(sg, av[0:64, :], recb2[:])
                        nc.scalar.dma_start(
                            out=a2a_in3[4 * b + c][hs, :], in_=_r(sg)
                        )

            # ---- all-to-all: heads -> sequence quarters ----
            nc.gpsimd.collective_compute(
                "AllToAll",
                mybir.AluOpType.bypass,
                replica_groups=[list(range(NCORES))],
                ins=[a2a_in[:]],
                outs=[a2a_out[:]],
            )

            # ---- out projection for my sequence quarter ----
            at2 = [at2p.tile([128, CHUNK], F32R, tag=f"at{g}", name=f"at{g}") for g in range(NCORES)]
            for g in range(NCORES):
                nc.scalar.dma_start(out=at2[g], in_=a2a_out3[g])
            for et in range(NKT):
                pm = ps.tile([128, CHUNK], F32, tag="mm", name="pm_yproj")
                for k in range(NKT):
                    nc.tensor.matmul(
                        pm, wo_t[k][:, 128*et:128*(et+1)], at2[k],
                        start=(k == 0), stop=(k == NKT - 1),
                    )
                ys = ysp.tile([128, CHUNK], F32, tag="ys")
                nc.scalar.activation(
                    ys, pm, mybir.ActivationFunctionType.Identity,
                    bias=bo_t[:, et:et+1],
                )
                nc.scalar.dma_start(out=ytq[128*et:128*(et+1), :], in_=ys)

    nc.finalize()
    return nc


_NC_CACHE = None


def _get_program():
    global _NC_CACHE
    if _NC_CACHE is None:
        _NC_CACHE = build_program()
    return _NC_CACHE


def _prep_in_maps(x, cos, sin, Wq, bq, Wk, bk, Wv, bv, Wo, bo):
    cosT = np.ascontiguousarray(cos.T).astype(np.float32)    # (32, S)
    sinT = np.ascontiguousarray(sin.T).astype(np.float32)
    chat = np.concatenate([cosT, cosT, cosT, cosT], 0)       # (128, S)
    shat = np.concatenate([-sinT, sinT, -sinT, sinT], 0)
    xT = [np.ascontiguousarray(x[b].T) for b in range(B)]
    mask128 = np.where(np.arange(128)[:, None] > np.arange(128)[None, :],
                       np.float32(MASKVAL), np.float32(0.0)).astype(np.float32)
    sw = np.arange(128); sw = np.where((sw // 32) % 2 == 0, sw + 32, sw - 32)
    perm128 = np.zeros((128, 128), np.float32)
    perm128[sw, np.arange(128)] = 1.0
    wqT, wkT, wvT, woT = (np.ascontiguousarray(W.T) for W in (Wq, Wk, Wv, Wo))

    in_maps = []
    for core in range(NCORES):
        sl = slice(F * core, F * (core + 1))
        in_maps.append({
            "xt0": xT[0], "xt1": xT[1],
            "wqt": np.ascontiguousarray(wqT[:, sl]),
            "wkt": np.ascontiguousarray(wkT[:, sl]),
            "wvt": np.ascontiguousarray(wvT[:, sl]),
            "bq": np.ascontiguousarray(bq[sl]),
            "bk": np.ascontiguousarray(bk[sl]),
            "bv": np.ascontiguousarray(bv[sl]),
            "wot": woT, "bo": bo,
            "chat": chat, "shat": shat,
            "ident128": np.eye(128, dtype=np.float32),
            "perm128": perm128,
            "ones64": np.ones((1, 64), np.float32),
            "vones": np.ones((NST, HPC), np.float32),
            "mask128": mask128,
        })
    return in_maps


def kernel(x, cos, sin, mask, Wq, bq, Wk, bk, Wv, bv, Wo, bo, **_unused):
    """Full inputs in, full output out. `mask` (the causal mask) is
    regenerated on-device, so the input tensor itself is unused."""
    x, cos, sin = (np.asarray(a, np.float32) for a in (x, cos, sin))
    Wq, bq, Wk, bk = (np.asarray(a, np.float32) for a in (Wq, bq, Wk, bk))
    Wv, bv, Wo, bo = (np.asarray(a, np.float32) for a in (Wv, bv, Wo, bo))

    nc = _get_program()
    in_maps = _prep_in_maps(x, cos, sin, Wq, bq, Wk, bk, Wv, bv, Wo, bo)

    trace = bool(int(os.environ.get("MHA_TRACE", "0")))
    kw = {}
    if trace:
        _install_ntff_hook()
        kw = dict(trace=True, trace_cores=list(range(NCORES)))
    res = run_bass_kernel_spmd(nc, in_maps, core_ids=list(range(NCORES)), **kw)
    kernel.last_results = res

    y = np.empty((B, S, D), np.float32)
    for r in range(NCORES):
        b, c = r // NCH, r % NCH
        y[b, CHUNK*c:CHUNK*(c+1), :] = res.results[r]["ytq"].T
    return y
